# revision 8
# baseline (speedup 1.0000x reference)
"""Causal self-attention (B=4, T=2048, C=1024, H=16, D=64) on 8 TRN2 NeuronCores.

Sharding: core c handles batch b = c//2 and head-group hg = c%2 (8 of 16 heads).
Per core: column-sharded QKV projection (only its heads' q/k/v columns, only its
batch's rows), full causal attention for its 8 heads, row-sharded output
projection producing a partial [T, C] result. Host sums the two head-group
partials per batch (the "all-reduce") and adds the bias correction term.

v2 restructure vs v1:
 - Software pipeline over 512-row query blocks: qkv-projection(q5+1) and
   out-projection(q5-1) are issued interleaved with attention(q5), so the
   PE keeps running while the ACT engine grinds through the exps (the
   attention phase is ACT-bound at ~150us).
 - Attention output stays in y^T layout end to end: the PV matmul produces
   y^T[65, q] directly (ones column of V' at index 64 -> row 64 of the PSUM
   accumulator is the softmax denominator).
   Normalization is a K=1 ones-outer-product broadcast of the denominator
   row on the PE plus one DVE divide, writing yT_sb. The out projection
   consumes yT_sb directly as lhsT. This removes both transpose rounds of
   v1 (y^T -> y natural -> y^T again).
 - DMA issue order: the first xt slab and the first-needed wqk column
   chunks go out first on separate rings so the first matmul starts ~7us
   in instead of ~24us.

Math notes:
 - k-bias is dropped: softmax((q+bq)@(k+bk)^T) == softmax((q+bq)@k^T).
 - v-bias and proj-bias are folded into a host-side correction: since
   softmax rows sum to 1, the correction is bv @ w_proj + b_proj.
 - exp uses no max subtraction (logits are ~N(0, 0.41); exp fits f32/bf16).
"""

import numpy as np
import ml_dtypes

B, T, C, H, D = 4, 2048, 1024, 16, 64
HC = 8            # heads per core
KCH = C // 128    # 8 contraction chunks
RC = T // 128     # 16 row chunks
QQ = T // 512     # 4 query super-blocks
BF16 = ml_dtypes.bfloat16

_COMPILED = {}


def _build_nc():
    from concourse import bacc
    import concourse.tile as tile
    from concourse import mybir

    bf16 = mybir.dt.bfloat16
    f32 = mybir.dt.float32
    EXP = mybir.ActivationFunctionType.Exp
    ADD = mybir.AluOpType.add
    MULT = mybir.AluOpType.mult

    nc = bacc.Bacc(None, target_bir_lowering=False)

    xT = nc.dram_tensor("xT", [128, KCH, T], bf16, kind="ExternalInput")
    # cc-major: [part, cc(0-3 q, 4-7 k), kc, 128]
    wqk = nc.dram_tensor("wqk", [128, 8, KCH, 128], bf16, kind="ExternalInput")
    wv = nc.dram_tensor("wv", [128, KCH, 512], bf16, kind="ExternalInput")
    bq = nc.dram_tensor("bq", [128, 4], f32, kind="ExternalInput")
    wp = nc.dram_tensor("wp", [128, 4, 1024], bf16, kind="ExternalInput")
    out = nc.dram_tensor("out", [T, C], f32, kind="ExternalOutput")

    # Causal mask for the diagonal 128-key x 512-q blocks, variant r = kc % 4:
    # valid iff r*128 + k <= q. Applied multiplicatively to exp(S) in bf16.
    kk = np.arange(128)[:, None, None]
    rr = np.arange(4)[None, :, None]
    qq = np.arange(512)[None, None, :]
    mask_np = (rr * 128 + kk <= qq).astype(BF16)
    msk = nc.inline_tensor(mask_np, name="msk")

    with tile.TileContext(nc) as tc:
        with tc.tile_pool(name="singles", bufs=1) as singles:
            wqk_sb = singles.tile([128, 8, KCH, 128], bf16)
            wv_sb = singles.tile([128, KCH, 512], bf16)
            bq_sb = singles.tile([128, 4], f32)
            wp_sb = singles.tile([128, 4, 1024], bf16)
            msk_sb = singles.tile([128, 4, 512], bf16)
            ones_sb = singles.tile([128, 64], bf16)
            nc.vector.memset(ones_sb[64:65, :], 1.0)

            # DMA order: first q-column weights + first x slab race in on
            # separate rings so cc=0 can start ASAP; the rest follows.
            nc.scalar.dma_start(wqk_sb[:, 0:2], wqk[:, 0:2])
            nc.scalar.dma_start(wqk_sb[:, 2:4], wqk[:, 2:4])
            nc.scalar.dma_start(wqk_sb[:, 4:6], wqk[:, 4:6])
            nc.scalar.dma_start(wqk_sb[:, 6:8], wqk[:, 6:8])
            nc.gpsimd.dma_start(wv_sb[:], wv[:])
            nc.gpsimd.dma_start(bq_sb[:], bq[:])
            nc.gpsimd.dma_start(msk_sb[:], msk[:])
            nc.gpsimd.dma_start(wp_sb[:], wp[:])

            # persistent activations
            qT_sb = singles.tile([128, 4, T], bf16)   # q^T, heads 2c,2c+1 in chunk c
            kT_sb = singles.tile([128, 4, T], bf16)
            v_sb = singles.tile([128, RC, HC, 65], bf16)  # V at 0:64, ones col at 64
            yT_sb = singles.tile([128, 4, T], bf16)       # y^T, normalized

            nc.vector.memset(v_sb[:, :, :, 64], 1.0)

            with tc.tile_pool(name="xt", bufs=2) as xp, \
                 tc.tile_pool(name="att", bufs=6) as ap, \
                 tc.tile_pool(name="ytf", bufs=4) as yp, \
                 tc.tile_pool(name="outp", bufs=3) as op_, \
                 tc.tile_pool(name="gen", bufs=2, space="PSUM") as gen, \
                 tc.tile_pool(name="psS", bufs=2, space="PSUM") as psS, \
                 tc.tile_pool(name="psY", bufs=2, space="PSUM") as psY:

                def p1(r5):
                    # QKV projection for rows [r5*512, (r5+1)*512)
                    sl = slice(r5 * 512, (r5 + 1) * 512)
                    xt = xp.tile([128, KCH, 512], bf16)
                    nc.sync.dma_start(xt[:], xT[:, :, sl])
                    for cc in range(8):       # qk column chunks (0-3 q, 4-7 k)
                        ps = gen.tile([128, 512], f32, name='g')
                        for kc in range(KCH):
                            nc.tensor.matmul(ps[:], wqk_sb[:, cc, kc, :],
                                             xt[:, kc, :],
                                             start=(kc == 0), stop=(kc == KCH - 1))
                        if cc < 4:
                            nc.vector.tensor_scalar(
                                out=qT_sb[:, cc, sl], in0=ps[:],
                                scalar1=bq_sb[:, cc:cc + 1], scalar2=0.125,
                                op0=ADD, op1=MULT)
                        else:
                            nc.vector.tensor_copy(out=kT_sb[:, cc - 4, sl], in_=ps[:])
                    for rs in range(4):       # v rows, 128 at a time
                        rc = r5 * 4 + rs
                        psv = gen.tile([128, 512], f32, name='g')
                        for kc in range(KCH):
                            nc.tensor.matmul(psv[:], xt[:, kc, rs * 128:(rs + 1) * 128],
                                             wv_sb[:, kc, :],
                                             start=(kc == 0), stop=(kc == KCH - 1))
                        nc.vector.tensor_copy(
                            out=v_sb[:, rc, :, 0:64],
                            in_=psv[:].rearrange("p (h d) -> p h d", h=HC))

                pend = []   # deferred normalizes: (ytf, head, q5)

                def _normalize(ytf, h, q5):
                    # y^T row 64 is the softmax denominator: reciprocal of
                    # that one row, K=1 matmul broadcast across 64
                    # partitions, then one multiply writes normalized y^T.
                    qsl = slice(q5 * 512, (q5 + 1) * 512)
                    i, pr = h % 2, h // 2
                    rb = yp.tile([128, 512], bf16, name="rcp")
                    with nc.allow_low_precision(reason="1/denom in bf16 is plenty"):
                        nc.vector.reciprocal(rb[64:65, :], ytf[64:65, :])
                    dbc = psY.tile([64, 512], f32, name="acc")
                    nc.tensor.matmul(dbc[:], ones_sb[64:65, :], rb[64:65, :],
                                     start=True, stop=True)
                    nc.vector.tensor_mul(
                        out=yT_sb[i * 64:(i + 1) * 64, pr, qsl],
                        in0=ytf[0:64, :], in1=dbc[:])

                def attn(q5):
                    # attention for queries [q5*512, (q5+1)*512), all 4 head pairs
                    qsl = slice(q5 * 512, (q5 + 1) * 512)
                    nkc = 4 * (q5 + 1)
                    for pr in range(4):       # head pair: local heads 2pr, 2pr+1
                        psyt = [psY.tile([65, 512], f32, name="acc")
                                for i in range(2)]
                        for kc in range(nkc):
                            diag = (kc // 4 == q5)
                            r = kc % 4
                            qof = r * 128 if diag else 0  # causal column trim
                            pss = psS.tile([128, 2, 512], f32)
                            for i in range(2):   # head in pair, packed in PE
                                po = i * 64
                                nc.tensor.matmul(
                                    pss[:, i, qof:],
                                    kT_sb[po:po + 64, pr, kc * 128:(kc + 1) * 128],
                                    qT_sb[po:po + 64, pr,
                                          q5 * 512 + qof:(q5 + 1) * 512],
                                    start=True, stop=True)
                            exps = ap.tile([128, 2, 512], bf16)
                            nc.scalar.activation(exps[:, :, qof:],
                                                 pss[:, :, qof:], EXP)
                            for i in range(2):
                                h = 2 * pr + i
                                if diag:
                                    # only the 128-col triangle needs masking;
                                    # columns past it are fully valid
                                    nc.vector.tensor_mul(
                                        out=exps[:, i, qof:qof + 128],
                                        in0=exps[:, i, qof:qof + 128],
                                        in1=msk_sb[:, r, qof:qof + 128])
                                nc.tensor.matmul(
                                    psyt[i][:, qof:], v_sb[:, kc, h, :],
                                    exps[:, i, qof:],
                                    start=(kc == 0), stop=(kc == nkc - 1))
                        # copy y^T (+denominator row 0) out of PSUM now;
                        # the normalize itself is deferred one pair so the
                        # PE isn't stalled waiting on these DVE copies
                        for i in range(2):
                            ytf = yp.tile([65, 512], bf16, name="ytf")
                            nc.vector.tensor_copy(out=ytf[:], in_=psyt[i][:])
                            pend.append((ytf, 2 * pr + i, q5))
                        while len(pend) > 2:
                            _normalize(*pend.pop(0))

                def p3(q5):
                    # output projection for rows [q5*512, (q5+1)*512)
                    while pend and pend[0][2] <= q5:
                        _normalize(*pend.pop(0))
                    for rs in range(4):
                        rc = q5 * 4 + rs
                        rsl = slice(rc * 128, (rc + 1) * 128)
                        for oh in range(2):
                            pso = gen.tile([128, 512], f32, name='g')
                            for fc in range(4):
                                nc.tensor.matmul(pso[:], yT_sb[:, fc, rsl],
                                                 wp_sb[:, fc, oh * 512:(oh + 1) * 512],
                                                 start=(fc == 0), stop=(fc == 3),
                                                 skip_group_check=True)
                            osb = op_.tile([128, 512], f32)
                            nc.vector.tensor_copy(out=osb[:], in_=pso[:])
                            ring = nc.scalar if (rc + oh) % 2 else nc.sync
                            ring.dma_start(out[rsl, oh * 512:(oh + 1) * 512], osb[:])

                # software pipeline: keep PE busy with projections while the
                # ACT engine works through attention exps
                p1(0)
                p1(1)
                attn(0)
                p1(2)
                attn(1)
                p3(0)
                p1(3)
                attn(2)
                p3(1)
                attn(3)
                p3(2)
                p3(3)

    nc.compile()
    return nc


def _prep_core_inputs(x, w_attn, b_attn, w_proj, c):
    b, hg = c // 2, c % 2
    xb = np.ascontiguousarray(x[b])                       # [T, C]
    xT = np.ascontiguousarray(
        xb.T.reshape(KCH, 128, T).transpose(1, 0, 2)).astype(BF16)
    wq = w_attn[:, hg * 512:(hg + 1) * 512]
    wk = w_attn[:, C + hg * 512:C + (hg + 1) * 512]
    wqk = np.concatenate([wq, wk], axis=1)                # [C, 1024]
    # [part, cc, kc, 128]
    wqk = np.ascontiguousarray(
        wqk.reshape(KCH, 128, 8, 128).transpose(1, 2, 0, 3)).astype(BF16)
    wv = w_attn[:, 2 * C + hg * 512:2 * C + (hg + 1) * 512]
    wv = np.ascontiguousarray(
        wv.reshape(KCH, 128, 512).transpose(1, 0, 2)).astype(BF16)
    bqv = np.ascontiguousarray(
        b_attn[hg * 512:(hg + 1) * 512].reshape(4, 128).T).astype(np.float32)
    wpc = w_proj[hg * 512:(hg + 1) * 512, :]
    wpc = np.ascontiguousarray(
        wpc.reshape(4, 128, 1024).transpose(1, 0, 2)).astype(BF16)
    return {"xT": xT, "wqk": wqk, "wv": wv, "bq": bqv, "wp": wpc}


def _run(nc, in_maps, **kwargs):
    from concourse.bass_utils import run_bass_kernel_spmd
    return run_bass_kernel_spmd(nc, in_maps, core_ids=list(range(8)), **kwargs)


def kernel(x, w_attn, b_attn, w_proj, b_proj, _trace=False):
    x = np.asarray(x, dtype=np.float32)
    w_attn = np.asarray(w_attn, dtype=np.float32)
    b_attn = np.asarray(b_attn, dtype=np.float32)
    w_proj = np.asarray(w_proj, dtype=np.float32)
    b_proj = np.asarray(b_proj, dtype=np.float32)

    if "nc" not in _COMPILED:
        _COMPILED["nc"] = _build_nc()
    nc = _COMPILED["nc"]

    in_maps = [_prep_core_inputs(x, w_attn, b_attn, w_proj, c) for c in range(8)]
    kwargs = {"trace": True} if _trace else {}
    res = _run(nc, in_maps, **kwargs)
    _COMPILED["last_result"] = res

    corr = b_attn[2 * C:].astype(np.float32) @ w_proj + b_proj
    out = np.empty((B, T, C), np.float32)
    for b in range(B):
        out[b] = res.results[2 * b]["out"] + res.results[2 * b + 1]["out"]
        out[b] += corr[None, :]
    return out


# revision 9
# speedup vs baseline: 1.2344x; 1.2344x over previous
"""Causal self-attention (B=4, T=2048, C=1024, H=16, D=64) on 8 TRN2 NeuronCores.

Sharding: core c handles batch b = c//2 and head-group hg = c%2 (8 of 16 heads).
Per core: column-sharded QKV projection (only its heads' q/k/v columns, only its
batch's rows), full causal attention for its 8 heads, row-sharded output
projection producing a partial [T, C] result. Host sums the two head-group
partials per batch (the "all-reduce") and adds the bias correction term.

v2 restructure vs v1:
 - Software pipeline over 512-row query blocks: qkv-projection(q5+1) and
   out-projection(q5-1) are issued interleaved with attention(q5), so the
   PE keeps running while the ACT engine grinds through the exps (the
   attention phase is ACT-bound at ~150us).
 - Attention output stays in y^T layout end to end: the PV matmul produces
   y^T[65, q] directly (ones column of V' at index 64 -> row 64 of the PSUM
   accumulator is the softmax denominator).
   Normalization is a K=1 ones-outer-product broadcast of the denominator
   row on the PE plus one DVE divide, writing yT_sb. The out projection
   consumes yT_sb directly as lhsT. This removes both transpose rounds of
   v1 (y^T -> y natural -> y^T again).
 - DMA issue order: the first xt slab and the first-needed wqk column
   chunks go out first on separate rings so the first matmul starts ~7us
   in instead of ~24us.

Math notes:
 - k-bias is dropped: softmax((q+bq)@(k+bk)^T) == softmax((q+bq)@k^T).
 - v-bias and proj-bias are folded into a host-side correction: since
   softmax rows sum to 1, the correction is bv @ w_proj + b_proj.
 - exp uses no max subtraction (logits are ~N(0, 0.41); exp fits f32/bf16).
"""

import numpy as np
import ml_dtypes

B, T, C, H, D = 4, 2048, 1024, 16, 64
HC = 8            # heads per core
KCH = C // 128    # 8 contraction chunks
RC = T // 128     # 16 row chunks
QQ = T // 512     # 4 query super-blocks
BF16 = ml_dtypes.bfloat16

_COMPILED = {}


def _build_nc():
    from concourse import bacc
    import concourse.tile as tile
    from concourse import mybir

    bf16 = mybir.dt.bfloat16
    f32 = mybir.dt.float32
    EXP = mybir.ActivationFunctionType.Exp
    ADD = mybir.AluOpType.add
    MULT = mybir.AluOpType.mult

    nc = bacc.Bacc(None, target_bir_lowering=False)

    xT = nc.dram_tensor("xT", [QQ, 128, KCH, 512], bf16, kind="ExternalInput")
    # cc-major: [part, cc(0-3 q, 4-7 k), kc, 128]
    wqk = nc.dram_tensor("wqk", [128, 8, KCH, 128], bf16, kind="ExternalInput")
    wv = nc.dram_tensor("wv", [128, KCH, 512], bf16, kind="ExternalInput")
    bq = nc.dram_tensor("bq", [128, 4], f32, kind="ExternalInput")
    wp = nc.dram_tensor("wp", [128, 4, 1024], bf16, kind="ExternalInput")
    out = nc.dram_tensor("out", [T, C], f32, kind="ExternalOutput")

    # Causal mask for the diagonal 128-key x 512-q blocks, variant r = kc % 4:
    # valid iff r*128 + k <= q. Applied multiplicatively to exp(S) in bf16.
    kk = np.arange(128)[:, None, None]
    rr = np.arange(4)[None, :, None]
    qq = np.arange(512)[None, None, :]
    mask_np = (rr * 128 + kk <= qq).astype(BF16)
    msk = nc.inline_tensor(mask_np, name="msk")

    with tile.TileContext(nc) as tc:
        with tc.tile_pool(name="singles", bufs=1) as singles:
            wqk_sb = singles.tile([128, 8, KCH, 128], bf16)
            wv_sb = singles.tile([128, KCH, 512], bf16)
            bq_sb = singles.tile([128, 4], f32)
            wp_sb = singles.tile([128, 4, 1024], bf16)
            msk_sb = singles.tile([128, 4, 512], bf16)
            ones_sb = singles.tile([128, 64], bf16)
            nc.vector.memset(ones_sb[64:65, :], 1.0)

            # DMA order: first q-column weights + first x slab race in on
            # separate rings so cc=0 can start ASAP; the rest follows.
            nc.scalar.dma_start(wqk_sb[:, 0:2], wqk[:, 0:2])
            nc.scalar.dma_start(wqk_sb[:, 2:4], wqk[:, 2:4])
            nc.scalar.dma_start(wqk_sb[:, 4:6], wqk[:, 4:6])
            nc.scalar.dma_start(wqk_sb[:, 6:8], wqk[:, 6:8])
            nc.gpsimd.dma_start(wv_sb[:], wv[:])
            nc.gpsimd.dma_start(bq_sb[:], bq[:])
            nc.gpsimd.dma_start(msk_sb[:], msk[:])
            nc.gpsimd.dma_start(wp_sb[:], wp[:])

            # persistent activations
            qT_sb = singles.tile([128, 4, T], bf16)   # q^T, heads 2c,2c+1 in chunk c
            kT_sb = singles.tile([128, 4, T], bf16)
            v_sb = singles.tile([128, RC, HC, 65], bf16)  # V at 0:64, ones col at 64
            yT_sb = singles.tile([128, 4, T], bf16)       # y^T, normalized

            nc.vector.memset(v_sb[:, :, :, 64], 1.0)

            with tc.tile_pool(name="xt", bufs=2) as xp, \
                 tc.tile_pool(name="att", bufs=6) as ap, \
                 tc.tile_pool(name="ytf", bufs=4) as yp, \
                 tc.tile_pool(name="outp", bufs=3) as op_, \
                 tc.tile_pool(name="gen", bufs=2, space="PSUM") as gen, \
                 tc.tile_pool(name="psS", bufs=2, space="PSUM") as psS, \
                 tc.tile_pool(name="psY", bufs=2, space="PSUM") as psY:

                def p1(r5):
                    # QKV projection for rows [r5*512, (r5+1)*512)
                    sl = slice(r5 * 512, (r5 + 1) * 512)
                    xt = xp.tile([128, KCH, 512], bf16)
                    nc.sync.dma_start(xt[:, 0:4], xT[r5, :, 0:4])
                    nc.scalar.dma_start(xt[:, 4:8], xT[r5, :, 4:8])
                    for cc in range(8):       # qk column chunks (0-3 q, 4-7 k)
                        ps = gen.tile([128, 512], f32, name='g')
                        for kc in range(KCH):
                            nc.tensor.matmul(ps[:], wqk_sb[:, cc, kc, :],
                                             xt[:, kc, :],
                                             start=(kc == 0), stop=(kc == KCH - 1))
                        if cc < 4:
                            nc.vector.tensor_scalar(
                                out=qT_sb[:, cc, sl], in0=ps[:],
                                scalar1=bq_sb[:, cc:cc + 1], scalar2=0.125,
                                op0=ADD, op1=MULT)
                        else:
                            nc.vector.tensor_copy(out=kT_sb[:, cc - 4, sl], in_=ps[:])
                    for rs in range(4):       # v rows, 128 at a time
                        rc = r5 * 4 + rs
                        psv = gen.tile([128, 512], f32, name='g')
                        for kc in range(KCH):
                            nc.tensor.matmul(psv[:], xt[:, kc, rs * 128:(rs + 1) * 128],
                                             wv_sb[:, kc, :],
                                             start=(kc == 0), stop=(kc == KCH - 1))
                        nc.vector.tensor_copy(
                            out=v_sb[:, rc, :, 0:64],
                            in_=psv[:].rearrange("p (h d) -> p h d", h=HC))

                pend = []   # deferred normalizes: (ytf, head, q5)

                def _normalize(ytf, h, q5):
                    # y^T row 64 is the softmax denominator: K=1 matmul
                    # broadcasts it across 64 partitions, one fast-approx
                    # reciprocal (~18 bits, plenty) on the broadcast, then
                    # one multiply writes normalized y^T.
                    qsl = slice(q5 * 512, (q5 + 1) * 512)
                    i, pr = h % 2, h // 2
                    dbc = psY.tile([64, 512], f32, name="acc")
                    nc.tensor.matmul(dbc[:], ones_sb[64:65, :], ytf[64:65, :],
                                     start=True, stop=True)
                    rb = yp.tile([64, 512], f32, name="rcp")
                    nc.vector.reciprocal_approx_fast(out=rb[:], in_=dbc[:])
                    nc.vector.tensor_mul(
                        out=yT_sb[i * 64:(i + 1) * 64, pr, qsl],
                        in0=ytf[0:64, :], in1=rb[:])

                def attn(q5):
                    # attention for queries [q5*512, (q5+1)*512), all 4 head pairs
                    qsl = slice(q5 * 512, (q5 + 1) * 512)
                    nkc = 4 * (q5 + 1)
                    for pr in range(4):       # head pair: local heads 2pr, 2pr+1
                        psyt = [psY.tile([65, 512], f32, name="acc")
                                for i in range(2)]
                        for kc in range(nkc):
                            diag = (kc // 4 == q5)
                            r = kc % 4
                            qof = r * 128 if diag else 0  # causal column trim
                            pss = psS.tile([128, 2, 512], f32)
                            for i in range(2):   # head in pair, packed in PE
                                po = i * 64
                                nc.tensor.matmul(
                                    pss[:, i, qof:],
                                    kT_sb[po:po + 64, pr, kc * 128:(kc + 1) * 128],
                                    qT_sb[po:po + 64, pr,
                                          q5 * 512 + qof:(q5 + 1) * 512],
                                    start=True, stop=True)
                            exps = ap.tile([128, 2, 512], bf16)
                            nc.scalar.activation(exps[:, :, qof:],
                                                 pss[:, :, qof:], EXP)
                            for i in range(2):
                                h = 2 * pr + i
                                if diag:
                                    # only the 128-col triangle needs masking;
                                    # columns past it are fully valid
                                    nc.vector.tensor_mul(
                                        out=exps[:, i, qof:qof + 128],
                                        in0=exps[:, i, qof:qof + 128],
                                        in1=msk_sb[:, r, qof:qof + 128])
                                nc.tensor.matmul(
                                    psyt[i][:, qof:], v_sb[:, kc, h, :],
                                    exps[:, i, qof:],
                                    start=(kc == 0), stop=(kc == nkc - 1))
                        # copy y^T (+denominator row 0) out of PSUM now;
                        # the normalize itself is deferred one pair so the
                        # PE isn't stalled waiting on these DVE copies
                        for i in range(2):
                            ytf = yp.tile([65, 512], bf16, name="ytf")
                            nc.vector.tensor_copy(out=ytf[:], in_=psyt[i][:])
                            pend.append((ytf, 2 * pr + i, q5))
                        while len(pend) > 2:
                            _normalize(*pend.pop(0))

                def p3(q5):
                    # output projection for rows [q5*512, (q5+1)*512)
                    while pend and pend[0][2] <= q5:
                        _normalize(*pend.pop(0))
                    for rs in range(4):
                        rc = q5 * 4 + rs
                        rsl = slice(rc * 128, (rc + 1) * 128)
                        for oh in range(2):
                            pso = gen.tile([128, 512], f32, name='g')
                            for fc in range(4):
                                nc.tensor.matmul(pso[:], yT_sb[:, fc, rsl],
                                                 wp_sb[:, fc, oh * 512:(oh + 1) * 512],
                                                 start=(fc == 0), stop=(fc == 3),
                                                 skip_group_check=True)
                            osb = op_.tile([128, 512], f32)
                            nc.vector.tensor_copy(out=osb[:], in_=pso[:])
                            ring = nc.scalar if (rc + oh) % 2 else nc.sync
                            ring.dma_start(out[rsl, oh * 512:(oh + 1) * 512], osb[:])

                # software pipeline: keep PE busy with projections while the
                # ACT engine works through attention exps
                p1(0)
                p1(1)
                attn(0)
                p1(2)
                attn(1)
                p3(0)
                p1(3)
                attn(2)
                p3(1)
                attn(3)
                p3(2)
                p3(3)

    nc.compile()
    return nc


def _prep_core_inputs(x, w_attn, b_attn, w_proj, c):
    b, hg = c // 2, c % 2
    xb = np.ascontiguousarray(x[b])                       # [T, C]
    xT = np.ascontiguousarray(
        xb.T.reshape(KCH, 128, QQ, 512).transpose(2, 1, 0, 3)).astype(BF16)
    wq = w_attn[:, hg * 512:(hg + 1) * 512]
    wk = w_attn[:, C + hg * 512:C + (hg + 1) * 512]
    wqk = np.concatenate([wq, wk], axis=1)                # [C, 1024]
    # [part, cc, kc, 128]
    wqk = np.ascontiguousarray(
        wqk.reshape(KCH, 128, 8, 128).transpose(1, 2, 0, 3)).astype(BF16)
    wv = w_attn[:, 2 * C + hg * 512:2 * C + (hg + 1) * 512]
    wv = np.ascontiguousarray(
        wv.reshape(KCH, 128, 512).transpose(1, 0, 2)).astype(BF16)
    bqv = np.ascontiguousarray(
        b_attn[hg * 512:(hg + 1) * 512].reshape(4, 128).T).astype(np.float32)
    wpc = w_proj[hg * 512:(hg + 1) * 512, :]
    wpc = np.ascontiguousarray(
        wpc.reshape(4, 128, 1024).transpose(1, 0, 2)).astype(BF16)
    return {"xT": xT, "wqk": wqk, "wv": wv, "bq": bqv, "wp": wpc}


def _run(nc, in_maps, **kwargs):
    from concourse.bass_utils import run_bass_kernel_spmd
    return run_bass_kernel_spmd(nc, in_maps, core_ids=list(range(8)), **kwargs)


def kernel(x, w_attn, b_attn, w_proj, b_proj, _trace=False):
    x = np.asarray(x, dtype=np.float32)
    w_attn = np.asarray(w_attn, dtype=np.float32)
    b_attn = np.asarray(b_attn, dtype=np.float32)
    w_proj = np.asarray(w_proj, dtype=np.float32)
    b_proj = np.asarray(b_proj, dtype=np.float32)

    if "nc" not in _COMPILED:
        _COMPILED["nc"] = _build_nc()
    nc = _COMPILED["nc"]

    in_maps = [_prep_core_inputs(x, w_attn, b_attn, w_proj, c) for c in range(8)]
    kwargs = {"trace": True} if _trace else {}
    res = _run(nc, in_maps, **kwargs)
    _COMPILED["last_result"] = res

    corr = b_attn[2 * C:].astype(np.float32) @ w_proj + b_proj
    out = np.empty((B, T, C), np.float32)
    for b in range(B):
        out[b] = res.results[2 * b]["out"] + res.results[2 * b + 1]["out"]
        out[b] += corr[None, :]
    return out
